# revision 2
# baseline (speedup 1.0000x reference)
"""CRF (emission matmul + logsumexp-semiring scan + gold path) on 8 TRN2 cores.

Strategy (hardcoded for T=16384, D=2048, K=16, 8 cores):
  - Shard the time axis: core c owns timesteps [c*2048, (c+1)*2048).
  - Host pre-tiles/casts seq -> fp8e4 [128, (quarter, d-chunk, t)] per core so
    each quarter's slab streams as two contiguous 0.5 MiB DMAs; weights are
    scaled x32 into fp8 range. All consts ride in one bf16 pack DMA.
  - Emission on PE: fp8 DoubleRow matmuls (contraction 256/step, 8 per
    quarter), PSUM f32. Raw emissions ship back as bf16 (host adds bias b).
  - Partition function via a parallel semiring scan in *linear* space: time is
    remapped so sub-chunk g = 256 qc + 32 sl + b covers t in [2g, 2g+2); the
    per-step scale table etab[(sl,i), qc, b, tau] = exp(emit) is produced by 8
    accumulating 0/1 selection matmuls on PE (exact partition expansion
    [16, 512] -> [128, 64] per quarter, no DMA) + exp on ACT.
  - Scan state [128=(sl,k), (b,j)] bf16: tau 0 is a pure broadcast multiply
    S_1 = Texp * e_t0 on DVE (no init matrix needed); tau 1 multiplies by the
    block-diagonal exp(transitions).T bf16 stationary on PE then scales on
    DVE. Four chains (one per quarter) interleave round-robin.
  - Cores ship emissions [16, 2048] bf16 and the 1024 linear-space sub-chunk
    matrices [128, 2048] bf16; the host combines 8192 16x16 matrices in f64
    (log-semiring) and computes the gold-path score from emissions.
"""

import numpy as np
import ml_dtypes

import concourse.bass as bass
import concourse.tile as tile
from concourse import bacc, mybir
from concourse.bass_utils import run_bass_kernel_spmd

BF16 = ml_dtypes.bfloat16
FP8 = ml_dtypes.float8_e4m3

T, D, K = 16384, 2048, 16
NCORES = 8
TC = T // NCORES            # 2048 timesteps per core
L = 2                       # scan steps per sub-chunk
NSL = 8                     # partition blocks of the scan state (s_l)
NBC = 128                   # column blocks of the scan state (b)
TBLK = 512                  # emission tile width (t)
NTB = TC // TBLK            # 4 quarters per core
NDCH = D // 128             # 16 contraction chunks
C_SHIFT = 3.3               # per-step log-space shift keeping f32 in range
QB = NBC // NTB             # 16 column blocks per quarter-chain
SCATTER_ENGINE = "gpsimd"   # "gpsimd" | "hwdge"
WSCALE = 32.0               # fp8 weight scale (emissions come out x WSCALE)


def _kernel_body(ctx, tc, seqt, wt, texp, bvec, init, emit_out, scan_out, reps=1):
    nc = tc.nc
    f32 = mybir.dt.float32
    bf16 = mybir.dt.bfloat16

    const_pool = ctx.enter_context(tc.tile_pool(name="const", bufs=1))
    seq_pool = ctx.enter_context(tc.tile_pool(name="seq", bufs=2))
    big_pool = ctx.enter_context(tc.tile_pool(name="big", bufs=2))
    state_pool = ctx.enter_context(tc.tile_pool(name="state", bufs=2))
    psum_e_pool = ctx.enter_context(tc.tile_pool(name="psum_e", bufs=2, space="PSUM"))
    psum_s_pool = ctx.enter_context(tc.tile_pool(name="psum_s", bufs=4, space="PSUM"))
    psum_x_pool = ctx.enter_context(tc.tile_pool(name="psum_x", bufs=2, space="PSUM"))

    # wt on SP first (first emission matmul needs it); other consts on ACT so
    # they don't delay the big seq loads behind them on the SP ring
    fp8 = mybir.dt.float8e4
    npack = 128 + 1 + NSL * 128 + NDCH * K + K
    pack_t = const_pool.tile([128, npack], bf16)
    nc.scalar.dma_start(out=pack_t[:], in_=texp)
    texp_t = pack_t[:, 0:128]
    sel_t = pack_t[0:K, 129:129 + NSL * 128]
    texps_t = pack_t[:, 129 + NSL * 128 + NDCH * K:]
    bmc_t = const_pool.tile([K, 1], f32)
    nc.vector.tensor_scalar_add(bmc_t[:], pack_t[0:K, 128:129], -C_SHIFT)
    wt_t = const_pool.tile([128, NDCH * K], fp8)
    nc.vector.tensor_copy(
        wt_t[:], pack_t[:, 129 + NSL * 128:129 + NSL * 128 + NDCH * K])

    pools = (seq_pool, big_pool, state_pool, psum_e_pool, psum_s_pool, psum_x_pool)
    consts = (wt_t, texp_t, bmc_t, texps_t, sel_t)
    if isinstance(reps, tuple):  # hardware loop for differential timing
        n_loop = reps[0]
        with tc.For_i(0, n_loop, 1, hint_engines=(
            mybir.EngineType.SP, mybir.EngineType.PE, mybir.EngineType.DVE,
        )):
            _rep_body(nc, tc, pools, consts, seqt, emit_out, scan_out)
        return
    for _rep in range(reps):
        _rep_body(nc, tc, pools, consts, seqt, emit_out, scan_out)


def _rep_body(nc, tc, pools, consts, seqt, emit_out, scan_out):
    seq_pool, big_pool, state_pool, psum_e_pool, psum_s_pool, psum_x_pool = pools
    wt_t, texp_t, bmc_t, texps_t, sel_t = consts
    f32 = mybir.dt.float32
    bf16 = mybir.dt.bfloat16
    fp8 = mybir.dt.float8e4

    emit_sb = big_pool.tile([K, TC], bf16, tag="emit_sb")
    exp_sb = big_pool.tile([K, TC], bf16, tag="exp_sb")
    etab = big_pool.tile([128, NTB, QB, L], bf16, tag="etab")
    fin_st = big_pool.tile([128, NBC * K], bf16, tag="fin_st")

    seqt_v = seqt.rearrange("p (q c t) -> p q c t", q=NTB, c=NDCH)
    # exp_sb viewed as [p, quarter, sl, 64]
    exp_v = exp_sb[:].rearrange("p (q sl c) -> p q sl c", q=NTB, sl=NSL)

    states = [None] * NTB

    # ---- all DMAs on the SP ring in explicit order: wt (gates the first
    # matmul), slab 0, the small consts, then the remaining slabs ----
    HD = NDCH // 2
    seq_tiles = []
    for q in range(NTB):
        sta = seq_pool.tile([128, HD, TBLK], fp8, tag=f"seqta{q}", name=f"seqta{q}")
        nc.sync.dma_start(out=sta[:], in_=seqt_v[:, q, 0:HD])
        stb = seq_pool.tile([128, HD, TBLK], fp8, tag=f"seqtb{q}", name=f"seqtb{q}")
        nc.sync.dma_start(out=stb[:], in_=seqt_v[:, q, HD:NDCH])
        seq_tiles.append((sta, stb))

    for q in range(NTB):
        tsl = bass.ts(q, TBLK)
        # ---- emission: psum[k, t] = sum_d W[k, d] * seq[t, d] ----
        pe = psum_e_pool.tile([K, TBLK], f32)
        for cp in range(NDCH // 2):
            half = seq_tiles[q][0] if cp < NDCH // 4 else seq_tiles[q][1]
            hc = 2 * cp if cp < NDCH // 4 else 2 * cp - NDCH // 2
            nc.tensor.matmul(
                pe[:],
                wt_t[:, 2 * cp * K:(2 * cp + 2) * K].rearrange(
                    "p (kt i) -> p kt i", kt=2),
                half[:, hc:hc + 2, :],
                start=(cp == 0),
                stop=(cp == NDCH // 2 - 1),
                perf_mode=mybir.MatmulPerfMode.DoubleRow,
            )
        # exp(emit + b - C_SHIFT) straight from PSUM -- first, it feeds the
        # scan-critical etab path
        nc.scalar.activation(
            out=exp_sb[:, tsl],
            in_=pe[:],
            func=mybir.ActivationFunctionType.Exp,
            bias=bmc_t[:],
            scale=1.0 / WSCALE,
        )
        # ---- partition expansion [16, (sl, 64)] -> [(sl, 16), 64] on PE:
        # 8 accumulating selection matmuls (0/1 weights, exact) ----
        etp = psum_x_pool.tile([128, QB * L], f32)
        for sl in range(NSL):
            nc.tensor.matmul(
                etp[:],
                sel_t[:, sl * 128:(sl + 1) * 128],
                exp_v[:, q, sl],
                start=(sl == 0),
                stop=(sl == NSL - 1),
            )
        nc.vector.tensor_copy(
            etab[:, q].rearrange("p b tau -> p (b tau)"), etp[:])
        # raw emissions out (host adds bias b); stores on SP so they never
        # block the ACT sequencer (exp) behind a not-yet-ready wait
        nc.vector.tensor_scalar_mul(emit_sb[:, tsl], pe[:], 1.0 / WSCALE)
        nc.sync.dma_start(out=emit_out[:, tsl], in_=emit_sb[:, tsl])

    # ---- scan: 4 chains of L steps. tau 0 is a pure broadcast multiply
    # (S_1[k, j] = Texp[k, j] * e_t0[k]) on DVE; later taus matmul + mul ----
    texps_b = texps_t.rearrange("p (o j) -> p o j", o=1).broadcast_to(
        [128, QB, K])
    for tau in range(L):
        pss = []
        for qc in range(NTB):
            if tau == 0:
                pss.append(None)
                continue
            ps = psum_s_pool.tile([128, QB * K], f32)
            nc.tensor.matmul(ps[:], texp_t[:, 0:128], states[qc],
                             start=True, stop=True)
            pss.append(ps)
        for qc in range(NTB):
            if tau == L - 1:
                newst = fin_st[:, qc * QB * K:(qc + 1) * QB * K]
            else:
                newst = state_pool.tile(
                    [128, QB * K], bf16, tag=f"st{qc}", name=f"st{qc}"
                )[:]
            src_ap = texps_b if tau == 0 else pss[qc][:].rearrange(
                "p (b j) -> p b j", b=QB)
            nc.vector.tensor_mul(
                newst.rearrange("p (b j) -> p b j", b=QB),
                src_ap,
                etab[:, qc, :, tau:tau + 1].broadcast_to([128, QB, K]),
            )
            states[qc] = newst
            if tau == L - 1:
                nc.sync.dma_start(
                    out=scan_out[:, qc * QB * K:(qc + 1) * QB * K],
                    in_=fin_st[:, qc * QB * K:(qc + 1) * QB * K],
                )


_PROGRAMS = {}


def _build_program(reps=1):
    if reps in _PROGRAMS:
        return _PROGRAMS[reps]
    from contextlib import ExitStack

    nc = bacc.Bacc(
        "TRN2", target_bir_lowering=False, debug=False, enable_asserts=False
    )
    f32 = mybir.dt.float32
    bf16 = mybir.dt.bfloat16
    seqt = nc.dram_tensor("seqt", [128, NTB * NDCH * TBLK], mybir.dt.float8e4, kind="ExternalInput")
    wt = None
    texp = nc.dram_tensor(
        "cpack", [128, 128 + 1 + NSL * 128 + NDCH * K + K], bf16,
        kind="ExternalInput")
    bvec = None
    init = None
    emit_out = nc.dram_tensor("emit_out", [K, TC], bf16, kind="ExternalOutput")
    scan_out = nc.dram_tensor("scan_out", [128, NBC * K], bf16, kind="ExternalOutput")

    with tile.TileContext(nc) as tc:
        with ExitStack() as ctx:
            _kernel_body(
                ctx, tc,
                seqt.ap(), None, texp.ap(), None, None,
                emit_out.ap(), scan_out.ap(), reps=reps,
            )
    nc.compile()
    _PROGRAMS[reps] = nc
    return nc


def _host_inputs(seq, W, b, transitions):
    """Build the per-core input maps (host-side preprocessing)."""
    seq8 = np.asarray(seq, dtype=np.float32).astype(FP8)        # [T, D]
    # wt[p, c*16+i] = W[i, c*128+p]
    wt = np.ascontiguousarray(
        (W * WSCALE).reshape(K, NDCH, 128).transpose(2, 1, 0).reshape(128, NDCH * K)
    ).astype(FP8)
    Texp = np.exp(transitions.astype(np.float64)).astype(np.float32)
    Thi = Texp.astype(BF16)
    texp_bd = np.zeros((128, 128), dtype=BF16)
    for s in range(NSL):
        texp_bd[s * K:(s + 1) * K, s * K:(s + 1) * K] = Thi.T
    cpack = np.zeros((128, 128 + 1 + NSL * 128 + NDCH * K + K), dtype=BF16)
    cpack[:, :128] = texp_bd
    cpack[:K, 128] = b.astype(BF16)
    sel = np.zeros((K, NSL * 128), dtype=BF16)
    for sl in range(NSL):
        for i in range(K):
            sel[i, sl * 128 + sl * K + i] = 1
    cpack[:K, 129:129 + NSL * 128] = sel
    cpack[:, 129 + NSL * 128:129 + NSL * 128 + NDCH * K] = wt.astype(BF16)
    cpack[:, 129 + NSL * 128 + NDCH * K:] = np.tile(Thi, (NSL, 1))
    in_maps = []
    for c in range(NCORES):
        seqc = seq8[c * TC:(c + 1) * TC]                         # [2048, 2048]
        st = np.ascontiguousarray(
            seqc.reshape(NTB, TBLK, NDCH, 128).transpose(3, 0, 2, 1)
        ).reshape(128, NTB * NDCH * TBLK)
        in_maps.append({
            "seqt": st,
            "cpack": cpack,
        })
    return in_maps


def _lse1(x):
    m = x.max(axis=1, keepdims=True)
    return (m + np.log(np.exp(x - m).sum(axis=1, keepdims=True)))[:, 0]


def _host_combine(emit, scan_mats, tags, b, trans_start, transitions, trans_end):
    """emit: [T, K] f32 raw W@x; scan_mats: [S, K, K] linear f32 (shift C/step)."""
    emit64 = emit.astype(np.float64) + b.astype(np.float64)[None, :]
    tr64 = transitions.astype(np.float64)
    tags = np.asarray(tags).astype(np.int64)

    alpha = trans_start.astype(np.float64) + emit64[0]
    for t in range(1, L):
        alpha = _lse1(tr64 + alpha[None, :]) + emit64[t]
    logM = np.log(np.maximum(scan_mats.astype(np.float64), 1e-300)) + L * C_SHIFT
    S = logM.shape[0]
    for g in range(1, S):
        alpha = _lse1(logM[g] + alpha[None, :])
    v = trans_end.astype(np.float64) + alpha
    log_z = v.max() + np.log(np.exp(v - v.max()).sum())

    gold = (
        trans_start.astype(np.float64)[tags[0]]
        + emit64[0, tags[0]]
        + tr64[tags[1:], tags[:-1]].sum()
        + emit64[np.arange(1, T), tags[1:]].sum()
        + trans_end.astype(np.float64)[tags[-1]]
    )
    return np.float32(gold - log_z)


def _run_device(in_maps, reps=1, **kwargs):
    nc = _build_program(reps)
    return run_bass_kernel_spmd(nc, in_maps, list(range(NCORES)), **kwargs)


def _decode_outputs(results):
    emit_parts = [np.asarray(results[c]["emit_out"]) for c in range(NCORES)]
    emit = np.concatenate(emit_parts, axis=1).T.astype(np.float32)  # [T, K]
    mats = []
    for c in range(NCORES):
        so = np.asarray(results[c]["scan_out"]).astype(np.float32)  # [128, 1024]
        # rows (sl, k), cols (b_col, j); g = 128*(b_col//16) + 16*sl + b_col%16
        m = (
            so.reshape(NSL, K, NTB, QB, K)
            .transpose(2, 0, 3, 1, 4)          # [q, sl, b_local, k, j]
            .reshape(-1, K, K)
        )
        mats.append(m)
    return emit, np.concatenate(mats, axis=0)  # [T,K], [4096, K, K]


def kernel(**inputs):
    seq = np.asarray(inputs["seq"], dtype=np.float32)
    tags = np.asarray(inputs["tags"])
    W = np.asarray(inputs["W"], dtype=np.float32)
    b = np.asarray(inputs["b"], dtype=np.float32)
    trans_start = np.asarray(inputs["trans_start"], dtype=np.float32)
    transitions = np.asarray(inputs["transitions"], dtype=np.float32)
    trans_end = np.asarray(inputs["trans_end"], dtype=np.float32)

    in_maps = _host_inputs(seq, W, b, transitions)
    results = _run_device(in_maps).results
    emit, scan_mats = _decode_outputs(results)
    return np.asarray(
        _host_combine(emit, scan_mats, tags, b, trans_start, transitions, trans_end)
    )
